# revision 15
# baseline (speedup 1.0000x reference)
"""Trainium2 Bass kernel for nn_BaseLoftqLinear (4-bit quantized linear + LoRA + bias).

Computes: out = x @ dequant(W).T + (x @ A.T) @ B.T + bias
  x: [4, 2048, 4096] f32, W: [4096, 4096] 4-bit packed, A: [16, 4096], B: [4096, 16]

Strategy (column-parallel over out_features across 8 cores, pure streaming GEMM):
  - each core owns 512 out_features; x replicated
  - host does ALL weight prep in f32 (unpack 4-bit, codebook lookup, per-block
    absmax scale, LoRA fold (B@A).T), rounds once to bf16, chunk-major layout
  - x -> bf16 k-major, re-tiled on host to [128p, g, q, 4*1024] so one
    partition row of an x tile is 4 k-chunks x 1024 m CONTIGUOUS = 8KB DMA
    lines.  The DMA fabric shares packet slots round-robin across queues, so
    bandwidth share is proportional to line size: 8KB lines everywhere.
  - out is partition-major [128, MT*N] so stores ship as 4-m-tile quads with
    8KB lines (2KB-line stores get starved and backpressure PSUM reuse);
    host un-permutes after gather
  - device: out[128m, 512n] += xT[128k,128m].T @ W[128k, 512n] over 32 k-chunks
    per m-tile; bias add on DVE; superblocks 0/1 run in chase mode (k-blocks
    4,4,8,16 emitted c-outer so every x tile / W group is needed as late as
    possible); superblocks 2..7 run k-inner with a superblock of prefetch
  - startup: x(0,0) split across the two earliest-starting queues
    (scalar+sync) gates the first matmul ~13us in
"""
import os
import sys

for _p in ("/opt/trn_rl_repo", "/root/.axon_site/_ro/trn_rl_repo"):
    if os.path.isdir(_p) and _p not in sys.path:
        sys.path.insert(0, _p)
        break

import numpy as np
import ml_dtypes

import concourse.bass as bass
import concourse.bacc as bacc
import concourse.tile as tile
import concourse.mybir as mybir

dt = mybir.dt

# problem constants (hardcoded per spec)
B_, S_, IN_F, OUT_F, RANK = 4, 2048, 4096, 4096, 16
M = B_ * S_                    # 8192 tokens
N_CORES = 8
N = OUT_F // N_CORES           # 512 out_features per core
BLOCK = 64                     # quant block size (along in_features)
NBLK = IN_F // BLOCK           # 64 scale blocks along k
MT = M // 128                  # 64 m-tiles
KC = IN_F // 128               # 32 k-chunks
KBLOCKS = [4, 4, 8, 16]        # k-run lengths for chase-mode superblocks
SB = 8                         # m-tiles per x superblock
NSB = MT // SB                 # 8 superblocks
XG = 4                         # k-chunks per x tile (8KB DMA lines)
NQ = KC // XG                  # 8 x tiles per superblock
XROW = NSB * KC * 1024         # elements per partition row of xt4


def build_program():
    """Single-core Bass program (SPMD: same program on all 8 cores)."""
    nc = bacc.Bacc("TRN2", target_bir_lowering=False, debug=False,
                   num_devices=N_CORES)

    # xt4[p, ((g*NQ + q)*XG + s)*1024 + mm] = x_bf16[g*1024 + mm,
    #                                                (q*XG + s)*128 + p]
    xt4 = nc.dram_tensor("xt4", [128, XROW], dt.bfloat16,
                         kind="ExternalInput")
    # W_eff chunk-major: weff[p, c*N+nn] = W_eff[c*128+p, nn]
    weff = nc.dram_tensor("weff", [128, KC * N], dt.bfloat16,
                          kind="ExternalInput")
    bias = nc.dram_tensor("bias", [N], dt.float32, kind="ExternalInput")
    # out partition-major: out2[p, ms*N+nn] = out[ms*128+p, nn]
    out = nc.dram_tensor("out", [128, MT * N], dt.float32,
                         kind="ExternalOutput")

    with tile.TileContext(nc) as tc:
        with (
            tc.tile_pool(name="const", bufs=1) as constp,
            tc.tile_pool(name="wt", bufs=1) as wtp,
            tc.tile_pool(name="xbig", bufs=16) as xbp,
            tc.tile_pool(name="oq", bufs=2) as oqp,
            tc.tile_pool(name="os1", bufs=4) as osp,
            tc.tile_pool(name="ps_out", bufs=8, space="PSUM") as ps_out,
        ):
            xt_t = xt4[:, :].tensor
            big_tiles = {}     # (g, q) -> [128, 4096] tile

            def x_big_dma(g, q, eng, eng2=None):
                """x tile: k-chunks XG*q..XG*q+3 x 1024 m of superblock g."""
                xtile = xbp.tile([128, XG * 1024], dt.bfloat16, tag="xb")
                off = (g * NQ + q) * (XG * 1024)
                if eng2 is None:
                    src = bass.AP(xt_t, off, [[XROW, 128], [1, XG * 1024]])
                    eng.dma_start(out=xtile[:], in_=src)
                else:
                    s0 = bass.AP(xt_t, off, [[XROW, 64], [1, XG * 1024]])
                    s1 = bass.AP(xt_t, off + 64 * XROW,
                                 [[XROW, 64], [1, XG * 1024]])
                    eng.dma_start(out=xtile[0:64, :], in_=s0)
                    eng2.dma_start(out=xtile[64:128, :], in_=s1)
                big_tiles[(g, q)] = xtile

            def x_slice(g, c, j):
                """lhsT [128k, 128m] for k-chunk c, local m-tile j."""
                ap = big_tiles[(g, c // XG)][:]
                off = (c % XG) * 1024 + j * 128
                return bass.AP(ap.tensor, ap.offset + off,
                               [list(ap.ap[0]), [1, 128]])

            # W_eff resident: wt_sb[:, c*N + nn] = W_eff[c*128 + p, nn]
            wt_sb = wtp.tile([128, KC * N], dt.bfloat16, name="wt_sb")
            bias_sb = constp.tile([128, N], dt.float32, name="bias_sb")

            def w_dma(c0, c1):
                nc.sync.dma_start(out=wt_sb[:, c0 * N:c1 * N],
                                  in_=weff[:, c0 * N:c1 * N])

            # ---- startup DMAs ----
            # gate for block 1 = x(0,0) + W[0:8); split half-partition so the
            # scalar and sync queue HEADS each carry half of the gate (a
            # [128,*] DMA is always 128 packet-slots, so only slot count and
            # queue position matter, not width)
            x_big_dma(0, 0, nc.scalar, nc.sync)
            nc.scalar.dma_start(out=wt_sb[0:64, 0:8 * N],
                                in_=weff[0:64, 0:8 * N])
            nc.sync.dma_start(out=wt_sb[64:128, 0:8 * N],
                              in_=weff[64:128, 0:8 * N])
            # gpsimd (latest-starting queue): bias + odd x tiles
            bsrc = bass.AP(bias[:].tensor, 0, [[0, 128], [1, N]])
            nc.gpsimd.dma_start(out=bias_sb[:], in_=bsrc)
            # W fully resident ASAP (reused 8x per chunk; x only once):
            # rest of W right behind the gate on sync
            w_dma(8, 16)
            w_dma(16, 24)
            w_dma(24, 32)
            # deadline-ordered phase-1 x: (0,1) half-split for latency,
            # then evens on scalar / odds on gpsimd
            x_big_dma(0, 1, nc.scalar, nc.gpsimd)
            for q in (3, 5, 7):
                x_big_dma(0, q, nc.gpsimd)
            for q in (2, 4, 6):
                x_big_dma(0, q, nc.scalar)
            for q in (0, 2, 4, 6):
                x_big_dma(1, q, nc.scalar)
            for q in (1, 3, 5, 7):
                x_big_dma(1, q, nc.gpsimd)
            for q in (0, 2, 4, 6):
                x_big_dma(2, q, nc.scalar)
            for q in (1, 3, 5, 7):
                x_big_dma(2, q, nc.gpsimd)

            # ---- stores: quads of m-tiles, 8KB lines, partition-major out
            quad = {}   # ms0 -> o_sb tile

            def store_add(ms, po):
                """DVE bias-add of one m-tile into its quad tile; flush the
                quad DMA when the 4th slice lands. Last 4 m-tiles: singles."""
                if ms >= MT - 4:
                    o1 = osp.tile([128, N], dt.float32, tag="o1")
                    nc.vector.tensor_tensor(o1[:], po[:], bias_sb[:],
                                            mybir.AluOpType.add)
                    eng = nc.scalar if ms % 2 else nc.sync
                    if ms == MT - 1:
                        nc.sync.dma_start(
                            out=bass.AP(out[:, :].tensor, ms * N,
                                        [[MT * N, 64], [1, N]]),
                            in_=o1[0:64, :])
                        nc.scalar.dma_start(
                            out=bass.AP(out[:, :].tensor, 64 * MT * N + ms * N,
                                        [[MT * N, 64], [1, N]]),
                            in_=o1[64:128, :])
                    else:
                        eng.dma_start(out=out[:, ms * N:(ms + 1) * N],
                                      in_=o1[:])
                    return
                ms0 = (ms // 4) * 4
                if ms0 not in quad:
                    oq_t = oqp.tile([128, 4 * N], dt.float32, tag="oq")
                    quad[ms0] = oq_t
                o_sb = quad[ms0]
                sl = ms - ms0
                nc.vector.tensor_tensor(o_sb[:, sl * N:(sl + 1) * N],
                                        po[:], bias_sb[:],
                                        mybir.AluOpType.add)
                if sl == 3:
                    nc.sync.dma_start(out=out[:, ms0 * N:(ms0 + 4) * N],
                                      in_=o_sb[:])
                    del quad[ms0]

            def chase_superblock(g):
                """All 8 m-tiles of superblock g, k-blocks c-outer/j-inner."""
                po_g = []
                for _j in range(SB):
                    po = ps_out.tile([128, N], dt.float32, tag="po")
                    po_g.append(po)
                e = 0
                for b in KBLOCKS:
                    for c in range(e, e + b):
                        for j in range(SB):
                            nc.tensor.matmul(
                                po_g[j][:],
                                x_slice(g, c, j),
                                wt_sb[:, c * N:(c + 1) * N],
                                start=(c == 0), stop=(c == KC - 1),
                            )
                    e += b
                for j in range(SB):
                    store_add(g * SB + j, po_g[j])

            chase_superblock(0)
            chase_superblock(1)

            # ---- tail: k-inner m-tiles with full-superblock prefetch ----
            rr = [nc.scalar, nc.gpsimd]
            for ms in range(2 * SB, MT):
                g, j = ms // SB, ms % SB
                if g + 1 < NSB and (g + 1, j) not in big_tiles:
                    x_big_dma(g + 1, j, rr[j % 2])
                po = ps_out.tile([128, N], dt.float32, tag="po")
                for c in range(KC):
                    nc.tensor.matmul(
                        po[:],
                        x_slice(g, c, j),
                        wt_sb[:, c * N:(c + 1) * N],
                        start=(c == 0), stop=(c == KC - 1),
                    )
                store_add(ms, po)

    nc.compile()
    return nc


_cache = {}


def _get_program(lookup_table=None):
    # program is independent of input values
    if "nc" not in _cache:
        _cache["nc"] = build_program()
    return _cache["nc"]


def make_in_maps(inputs: dict):
    x = np.asarray(inputs["x"], dtype=np.float32).reshape(M, IN_F)
    xb = x.astype(ml_dtypes.bfloat16)
    # xt4[p, g, q, s, mm] = x[g*1024+mm, (q*XG+s)*128+p]
    xt4 = np.ascontiguousarray(
        xb.reshape(NSB, 1024, NQ, XG, 128).transpose(4, 0, 2, 3, 1)
    ).reshape(128, XROW)

    lut = np.asarray(inputs["lookup_table"], dtype=np.float32)
    pk_full = np.asarray(inputs["packed_qweight"]).astype(np.uint8).reshape(-1)
    idx_full = np.empty(pk_full.size * 2, np.uint8)
    idx_full[0::2] = pk_full & 15
    idx_full[1::2] = pk_full >> 4
    idx_full = idx_full.reshape(OUT_F, IN_F)

    wmax_full = np.asarray(inputs["weight_max"], dtype=np.float32).reshape(OUT_F, NBLK)
    lora_a = np.asarray(inputs["lora_A"], dtype=np.float32)
    lora_b = np.asarray(inputs["lora_B"], dtype=np.float32)
    bias_full = np.asarray(inputs["bias"], dtype=np.float32).reshape(-1)

    # full weight prep in f32, one rounding to bf16 at the end
    wf = lut[idx_full]                                    # [OUT_F, IN_F]
    wf = wf.reshape(OUT_F, NBLK, BLOCK) * wmax_full[:, :, None]
    wf = wf.reshape(OUT_F, IN_F) + lora_b @ lora_a        # LoRA fold

    in_maps = []
    for i in range(N_CORES):
        o0, o1 = i * N, (i + 1) * N
        wt = wf[o0:o1, :].T                               # [IN_F, N]
        weff = np.ascontiguousarray(
            wt.reshape(KC, 128, N).transpose(1, 0, 2).reshape(128, KC * N)
        ).astype(ml_dtypes.bfloat16)
        in_maps.append({
            "xt4": xt4,
            "weff": weff,
            "bias": bias_full[o0:o1],
        })
    return in_maps


def kernel(**inputs) -> np.ndarray:
    from concourse.bass_utils import run_bass_kernel_spmd

    nc = _get_program()
    in_maps = make_in_maps(inputs)
    res = run_bass_kernel_spmd(nc, in_maps, core_ids=list(range(N_CORES)))
    outs = []
    for r in res.results:
        o2 = np.asarray(r["out"], dtype=np.float32)       # [128, MT*N]
        outs.append(o2.reshape(128, MT, N).transpose(1, 0, 2).reshape(M, N))
    full = np.concatenate(outs, axis=1)  # [M, OUT_F]
    return full.reshape(B_, S_, OUT_F)


# revision 16
# speedup vs baseline: 1.1870x; 1.1870x over previous
"""Trainium2 Bass kernel for nn_BaseLoftqLinear (4-bit quantized linear + LoRA + bias).

Computes: out = x @ dequant(W).T + (x @ A.T) @ B.T + bias
  x: [4, 2048, 4096] f32, W: [4096, 4096] 4-bit packed, A: [16, 4096], B: [4096, 16]

Strategy (column-parallel over out_features across 8 cores, pure streaming GEMM):
  - each core owns 512 out_features; x replicated
  - host does ALL weight prep in f32 (unpack 4-bit, codebook lookup, per-block
    absmax scale, LoRA fold (B@A).T), rounds once to bf16, chunk-major layout
  - x -> bf16 k-major, re-tiled on host to [128p, g, q, 4*1024] so one
    partition row of an x tile is 4 k-chunks x 1024 m CONTIGUOUS = 8KB DMA
    lines.  The DMA fabric shares packet slots round-robin across queues, so
    bandwidth share is proportional to line size: 8KB lines everywhere.
  - out is partition-major [128, MT*N] so stores ship as 4-m-tile quads with
    8KB lines (2KB-line stores get starved and backpressure PSUM reuse);
    host un-permutes after gather
  - device: out[128m, 512n] += xT[128k,128m].T @ W[128k, 512n] over 32 k-chunks
    per m-tile; bias add on DVE; superblocks 0/1 run in chase mode (k-blocks
    4,4,8,16 emitted c-outer so every x tile / W group is needed as late as
    possible); superblocks 2..7 run k-inner with a superblock of prefetch
  - startup: x(0,0) split across the two earliest-starting queues
    (scalar+sync) gates the first matmul ~13us in
"""
import os
import sys

for _p in ("/opt/trn_rl_repo", "/root/.axon_site/_ro/trn_rl_repo"):
    if os.path.isdir(_p) and _p not in sys.path:
        sys.path.insert(0, _p)
        break

import numpy as np
import ml_dtypes

import concourse.bass as bass
import concourse.bacc as bacc
import concourse.tile as tile
import concourse.mybir as mybir

dt = mybir.dt

# problem constants (hardcoded per spec)
B_, S_, IN_F, OUT_F, RANK = 4, 2048, 4096, 4096, 16
M = B_ * S_                    # 8192 tokens
N_CORES = 8
N = OUT_F // N_CORES           # 512 out_features per core
BLOCK = 64                     # quant block size (along in_features)
NBLK = IN_F // BLOCK           # 64 scale blocks along k
MT = M // 128                  # 64 m-tiles
KC = IN_F // 128               # 32 k-chunks
KBLOCKS = [4, 4, 8, 16]        # k-run lengths for chase-mode superblocks
SB = 8                         # m-tiles per x superblock
NSB = MT // SB                 # 8 superblocks
XG = 4                         # k-chunks per x tile (8KB DMA lines)
NQ = KC // XG                  # 8 x tiles per superblock
XROW = NSB * KC * 1024         # elements per partition row of xt4


def build_program():
    """Single-core Bass program (SPMD: same program on all 8 cores)."""
    nc = bacc.Bacc("TRN2", target_bir_lowering=False, debug=False,
                   num_devices=N_CORES)

    # xt4[p, ((g*NQ + q)*XG + s)*1024 + mm] = x_bf16[g*1024 + mm,
    #                                                (q*XG + s)*128 + p]
    xt4 = nc.dram_tensor("xt4", [128, XROW], dt.bfloat16,
                         kind="ExternalInput")
    # W_eff chunk-major: weff[p, c*N+nn] = W_eff[c*128+p, nn]
    weff = nc.dram_tensor("weff", [128, KC * N], dt.bfloat16,
                          kind="ExternalInput")
    bias = nc.dram_tensor("bias", [N], dt.float32, kind="ExternalInput")
    # out partition-major: out2[p, ms*N+nn] = out[ms*128+p, nn]
    out = nc.dram_tensor("out", [128, MT * N], dt.float32,
                         kind="ExternalOutput")

    with tile.TileContext(nc) as tc:
        with (
            tc.tile_pool(name="const", bufs=1) as constp,
            tc.tile_pool(name="wt", bufs=1) as wtp,
            tc.tile_pool(name="xbig", bufs=16) as xbp,
            tc.tile_pool(name="oq", bufs=2) as oqp,
            tc.tile_pool(name="os1", bufs=4) as osp,
            tc.tile_pool(name="ps_out", bufs=8, space="PSUM") as ps_out,
        ):
            xt_t = xt4[:, :].tensor
            big_tiles = {}     # (g, q) -> [128, 4096] tile

            def x_big_dma(g, q, eng, eng2=None):
                """x tile: k-chunks XG*q..XG*q+3 x 1024 m of superblock g."""
                xtile = xbp.tile([128, XG * 1024], dt.bfloat16, tag="xb")
                off = (g * NQ + q) * (XG * 1024)
                if eng2 is None:
                    src = bass.AP(xt_t, off, [[XROW, 128], [1, XG * 1024]])
                    eng.dma_start(out=xtile[:], in_=src)
                else:
                    s0 = bass.AP(xt_t, off, [[XROW, 64], [1, XG * 1024]])
                    s1 = bass.AP(xt_t, off + 64 * XROW,
                                 [[XROW, 64], [1, XG * 1024]])
                    eng.dma_start(out=xtile[0:64, :], in_=s0)
                    eng2.dma_start(out=xtile[64:128, :], in_=s1)
                big_tiles[(g, q)] = xtile

            def x_slice(g, c, j):
                """lhsT [128k, 128m] for k-chunk c, local m-tile j."""
                ap = big_tiles[(g, c // XG)][:]
                off = (c % XG) * 1024 + j * 128
                return bass.AP(ap.tensor, ap.offset + off,
                               [list(ap.ap[0]), [1, 128]])

            # W_eff resident: wt_sb[:, c*N + nn] = W_eff[c*128 + p, nn]
            wt_sb = wtp.tile([128, KC * N], dt.bfloat16, name="wt_sb")
            bias_sb = constp.tile([128, N], dt.float32, name="bias_sb")

            def w_dma(c0, c1):
                nc.sync.dma_start(out=wt_sb[:, c0 * N:c1 * N],
                                  in_=weff[:, c0 * N:c1 * N])

            # ---- startup DMAs ----
            # gate for block 1 = x(0,0) + W[0:8); split half-partition so the
            # scalar and sync queue HEADS each carry half of the gate (a
            # [128,*] DMA is always 128 packet-slots, so only slot count and
            # queue position matter, not width)
            x_big_dma(0, 0, nc.scalar, nc.sync)
            nc.scalar.dma_start(out=wt_sb[0:64, 0:8 * N],
                                in_=weff[0:64, 0:8 * N])
            nc.sync.dma_start(out=wt_sb[64:128, 0:8 * N],
                              in_=weff[64:128, 0:8 * N])
            # deadline-ordered phase-1 x; gpsimd (latest-starting, slowest
            # queue) gets only slack-deadline odd tiles
            x_big_dma(0, 1, nc.scalar)
            x_big_dma(0, 2, nc.sync)
            for q in (3, 5, 7):
                x_big_dma(0, q, nc.gpsimd)
            for q in (4, 6):
                x_big_dma(0, q, nc.scalar)
            w_dma(8, 16)
            w_dma(16, 24)
            w_dma(24, 32)
            bsrc = bass.AP(bias[:].tensor, 0, [[0, 128], [1, N]])
            nc.scalar.dma_start(out=bias_sb[:], in_=bsrc)
            for q in (0, 2, 4, 6):
                x_big_dma(1, q, nc.scalar)
            for q in (1, 3, 5, 7):
                x_big_dma(1, q, nc.gpsimd)
            for q in (0, 2, 4, 6):
                x_big_dma(2, q, nc.scalar)
            for q in (1, 3, 5, 7):
                x_big_dma(2, q, nc.gpsimd)

            # ---- stores: quads of m-tiles, 8KB lines, partition-major out
            quad = {}   # ms0 -> o_sb tile

            def store_add(ms, po):
                """DVE bias-add of one m-tile into its quad tile; flush the
                quad DMA when the 4th slice lands. Last 4 m-tiles: singles."""
                if ms >= MT - 4:
                    o1 = osp.tile([128, N], dt.float32, tag="o1")
                    nc.vector.tensor_tensor(o1[:], po[:], bias_sb[:],
                                            mybir.AluOpType.add)
                    eng = nc.scalar if ms % 2 else nc.sync
                    if ms == MT - 1:
                        nc.sync.dma_start(
                            out=bass.AP(out[:, :].tensor, ms * N,
                                        [[MT * N, 64], [1, N]]),
                            in_=o1[0:64, :])
                        nc.scalar.dma_start(
                            out=bass.AP(out[:, :].tensor, 64 * MT * N + ms * N,
                                        [[MT * N, 64], [1, N]]),
                            in_=o1[64:128, :])
                    else:
                        eng.dma_start(out=out[:, ms * N:(ms + 1) * N],
                                      in_=o1[:])
                    return
                ms0 = (ms // 4) * 4
                if ms0 not in quad:
                    oq_t = oqp.tile([128, 4 * N], dt.float32, tag="oq")
                    quad[ms0] = oq_t
                o_sb = quad[ms0]
                sl = ms - ms0
                nc.vector.tensor_tensor(o_sb[:, sl * N:(sl + 1) * N],
                                        po[:], bias_sb[:],
                                        mybir.AluOpType.add)
                if sl == 3:
                    nc.sync.dma_start(out=out[:, ms0 * N:(ms0 + 4) * N],
                                      in_=o_sb[:])
                    del quad[ms0]

            def chase_superblock(g):
                """All 8 m-tiles of superblock g, k-blocks c-outer/j-inner."""
                po_g = []
                for _j in range(SB):
                    po = ps_out.tile([128, N], dt.float32, tag="po")
                    po_g.append(po)
                e = 0
                for b in KBLOCKS:
                    for c in range(e, e + b):
                        for j in range(SB):
                            nc.tensor.matmul(
                                po_g[j][:],
                                x_slice(g, c, j),
                                wt_sb[:, c * N:(c + 1) * N],
                                start=(c == 0), stop=(c == KC - 1),
                            )
                    e += b
                for j in range(SB):
                    store_add(g * SB + j, po_g[j])

            chase_superblock(0)
            chase_superblock(1)

            # ---- tail: k-inner m-tiles with full-superblock prefetch ----
            rr = [nc.scalar, nc.gpsimd]
            for ms in range(2 * SB, MT):
                g, j = ms // SB, ms % SB
                if g + 1 < NSB and (g + 1, j) not in big_tiles:
                    x_big_dma(g + 1, j, rr[j % 2])
                po = ps_out.tile([128, N], dt.float32, tag="po")
                for c in range(KC):
                    nc.tensor.matmul(
                        po[:],
                        x_slice(g, c, j),
                        wt_sb[:, c * N:(c + 1) * N],
                        start=(c == 0), stop=(c == KC - 1),
                    )
                store_add(ms, po)

    nc.compile()
    return nc


_cache = {}


def _get_program(lookup_table=None):
    # program is independent of input values
    if "nc" not in _cache:
        _cache["nc"] = build_program()
    return _cache["nc"]


def make_in_maps(inputs: dict):
    x = np.asarray(inputs["x"], dtype=np.float32).reshape(M, IN_F)
    xb = x.astype(ml_dtypes.bfloat16)
    # xt4[p, g, q, s, mm] = x[g*1024+mm, (q*XG+s)*128+p]
    xt4 = np.ascontiguousarray(
        xb.reshape(NSB, 1024, NQ, XG, 128).transpose(4, 0, 2, 3, 1)
    ).reshape(128, XROW)

    lut = np.asarray(inputs["lookup_table"], dtype=np.float32)
    pk_full = np.asarray(inputs["packed_qweight"]).astype(np.uint8).reshape(-1)
    idx_full = np.empty(pk_full.size * 2, np.uint8)
    idx_full[0::2] = pk_full & 15
    idx_full[1::2] = pk_full >> 4
    idx_full = idx_full.reshape(OUT_F, IN_F)

    wmax_full = np.asarray(inputs["weight_max"], dtype=np.float32).reshape(OUT_F, NBLK)
    lora_a = np.asarray(inputs["lora_A"], dtype=np.float32)
    lora_b = np.asarray(inputs["lora_B"], dtype=np.float32)
    bias_full = np.asarray(inputs["bias"], dtype=np.float32).reshape(-1)

    # full weight prep in f32, one rounding to bf16 at the end
    wf = lut[idx_full]                                    # [OUT_F, IN_F]
    wf = wf.reshape(OUT_F, NBLK, BLOCK) * wmax_full[:, :, None]
    wf = wf.reshape(OUT_F, IN_F) + lora_b @ lora_a        # LoRA fold

    in_maps = []
    for i in range(N_CORES):
        o0, o1 = i * N, (i + 1) * N
        wt = wf[o0:o1, :].T                               # [IN_F, N]
        weff = np.ascontiguousarray(
            wt.reshape(KC, 128, N).transpose(1, 0, 2).reshape(128, KC * N)
        ).astype(ml_dtypes.bfloat16)
        in_maps.append({
            "xt4": xt4,
            "weff": weff,
            "bias": bias_full[o0:o1],
        })
    return in_maps


def kernel(**inputs) -> np.ndarray:
    from concourse.bass_utils import run_bass_kernel_spmd

    nc = _get_program()
    in_maps = make_in_maps(inputs)
    res = run_bass_kernel_spmd(nc, in_maps, core_ids=list(range(N_CORES)))
    outs = []
    for r in res.results:
        o2 = np.asarray(r["out"], dtype=np.float32)       # [128, MT*N]
        outs.append(o2.reshape(128, MT, N).transpose(1, 0, 2).reshape(M, N))
    full = np.concatenate(outs, axis=1)  # [M, OUT_F]
    return full.reshape(B_, S_, OUT_F)


# revision 18
# speedup vs baseline: 1.1926x; 1.0047x over previous
"""Trainium2 Bass kernel for nn_BaseLoftqLinear (4-bit quantized linear + LoRA + bias).

Computes: out = x @ dequant(W).T + (x @ A.T) @ B.T + bias
  x: [4, 2048, 4096] f32, W: [4096, 4096] 4-bit packed, A: [16, 4096], B: [4096, 16]

Strategy (column-parallel over out_features across 8 cores, pure streaming GEMM):
  - each core owns 512 out_features; x replicated
  - host does ALL weight prep in f32 (unpack 4-bit, codebook lookup, per-block
    absmax scale, LoRA fold (B@A).T), rounds once to bf16, chunk-major layout
  - x -> bf16 k-major, re-tiled on host to [128p, g, q, 4*1024] so one
    partition row of an x tile is 4 k-chunks x 1024 m CONTIGUOUS = 8KB DMA
    lines.  The DMA fabric shares packet slots round-robin across queues, so
    bandwidth share is proportional to line size: 8KB lines everywhere.
  - out is partition-major [128, MT*N] so stores ship as 4-m-tile quads with
    8KB lines (2KB-line stores get starved and backpressure PSUM reuse);
    host un-permutes after gather
  - device: out[128m, 512n] += xT[128k,128m].T @ W[128k, 512n] over 32 k-chunks
    per m-tile; bias add on DVE; superblocks 0/1 run in chase mode (k-blocks
    4,4,8,16 emitted c-outer so every x tile / W group is needed as late as
    possible); superblocks 2..7 run k-inner with a superblock of prefetch
  - startup: x(0,0) split across the two earliest-starting queues
    (scalar+sync) gates the first matmul ~13us in
"""
import os
import sys

for _p in ("/opt/trn_rl_repo", "/root/.axon_site/_ro/trn_rl_repo"):
    if os.path.isdir(_p) and _p not in sys.path:
        sys.path.insert(0, _p)
        break

import numpy as np
import ml_dtypes

import concourse.bass as bass
import concourse.bacc as bacc
import concourse.tile as tile
import concourse.mybir as mybir

dt = mybir.dt

# problem constants (hardcoded per spec)
B_, S_, IN_F, OUT_F, RANK = 4, 2048, 4096, 4096, 16
M = B_ * S_                    # 8192 tokens
N_CORES = 8
N = OUT_F // N_CORES           # 512 out_features per core
BLOCK = 64                     # quant block size (along in_features)
NBLK = IN_F // BLOCK           # 64 scale blocks along k
MT = M // 128                  # 64 m-tiles
KC = IN_F // 128               # 32 k-chunks
KBLOCKS = [4, 4, 8, 16]        # k-run lengths for chase-mode superblocks
SB = 8                         # m-tiles per x superblock
NSB = MT // SB                 # 8 superblocks
XG = 4                         # k-chunks per x tile (8KB DMA lines)
NQ = KC // XG                  # 8 x tiles per superblock
XROW = NSB * KC * 1024         # elements per partition row of xt4


def build_program():
    """Single-core Bass program (SPMD: same program on all 8 cores)."""
    nc = bacc.Bacc("TRN2", target_bir_lowering=False, debug=False,
                   num_devices=N_CORES)

    # xt4[p, ((g*NQ + q)*XG + s)*1024 + mm] = x_bf16[g*1024 + mm,
    #                                                (q*XG + s)*128 + p]
    xt4 = nc.dram_tensor("xt4", [128, XROW], dt.bfloat16,
                         kind="ExternalInput")
    # W_eff chunk-major: weff[p, c*N+nn] = W_eff[c*128+p, nn]
    weff = nc.dram_tensor("weff", [128, KC * N], dt.bfloat16,
                          kind="ExternalInput")
    bias = nc.dram_tensor("bias", [N], dt.float32, kind="ExternalInput")
    # out partition-major: out2[p, ms*N+nn] = out[ms*128+p, nn]
    out = nc.dram_tensor("out", [128, MT * N], dt.float32,
                         kind="ExternalOutput")

    with tile.TileContext(nc) as tc:
        with (
            tc.tile_pool(name="const", bufs=1) as constp,
            tc.tile_pool(name="gsync", bufs=1) as gsp,
            tc.tile_pool(name="wt", bufs=1) as wtp,
            tc.tile_pool(name="xbig", bufs=16) as xbp,
            tc.tile_pool(name="oq", bufs=2) as oqp,
            tc.tile_pool(name="os1", bufs=4) as osp,
            tc.tile_pool(name="ps_out", bufs=8, space="PSUM") as ps_out,
        ):
            xt_t = xt4[:, :].tensor
            big_tiles = {}     # (g, q) -> [128, 4096] tile

            def x_big_dma(g, q, eng, eng2=None):
                """x tile: k-chunks XG*q..XG*q+3 x 1024 m of superblock g."""
                xtile = xbp.tile([128, XG * 1024], dt.bfloat16, tag="xb")
                off = (g * NQ + q) * (XG * 1024)
                if eng2 is None:
                    src = bass.AP(xt_t, off, [[XROW, 128], [1, XG * 1024]])
                    eng.dma_start(out=xtile[:], in_=src)
                else:
                    s0 = bass.AP(xt_t, off, [[XROW, 64], [1, XG * 1024]])
                    s1 = bass.AP(xt_t, off + 64 * XROW,
                                 [[XROW, 64], [1, XG * 1024]])
                    eng.dma_start(out=xtile[0:64, :], in_=s0)
                    eng2.dma_start(out=xtile[64:128, :], in_=s1)
                big_tiles[(g, q)] = xtile

            def x_slice(g, c, j):
                """lhsT [128k, 128m] for k-chunk c, local m-tile j."""
                ap = big_tiles[(g, c // XG)][:]
                off = (c % XG) * 1024 + j * 128
                return bass.AP(ap.tensor, ap.offset + off,
                               [list(ap.ap[0]), [1, 128]])

            # W_eff resident: wt_sb[:, c*N + nn] = W_eff[c*128 + p, nn]
            wt_sb = wtp.tile([128, KC * N], dt.bfloat16, name="wt_sb")
            bias_sb = constp.tile([128, N], dt.float32, name="bias_sb")

            def w_dma(c0, c1):
                nc.sync.dma_start(out=wt_sb[:, c0 * N:c1 * N],
                                  in_=weff[:, c0 * N:c1 * N])

            # ---- startup DMAs ----
            # gate for block 1 = x(0,0) + W[0:8); split half-partition so the
            # scalar and sync queue HEADS each carry half of the gate (a
            # [128,*] DMA is always 128 packet-slots, so only slot count and
            # queue position matter, not width)
            x_big_dma(0, 0, nc.scalar, nc.sync)
            nc.scalar.dma_start(out=wt_sb[0:64, 0:8 * N],
                                in_=weff[0:64, 0:8 * N])
            nc.sync.dma_start(out=wt_sb[64:128, 0:8 * N],
                              in_=weff[64:128, 0:8 * N])
            # idle the gpsimd queue until the gate lands (so the gate gets
            # all DMA slots): a tiny copy depending on the last gate column
            gate_sync = gsp.tile([128, 8], dt.bfloat16, name="gate_sync")
            nc.gpsimd.tensor_copy(gate_sync[:], wt_sb[:, 8 * N - 8:8 * N])
            # deadline-ordered phase-1 x; gpsimd (latest-starting, slowest
            # queue) gets only slack-deadline odd tiles
            x_big_dma(0, 1, nc.scalar)
            x_big_dma(0, 2, nc.sync)
            for q in (3, 5, 7):
                x_big_dma(0, q, nc.gpsimd)
            for q in (4, 6):
                x_big_dma(0, q, nc.scalar)
            w_dma(8, 16)
            w_dma(16, 24)
            w_dma(24, 32)
            bsrc = bass.AP(bias[:].tensor, 0, [[0, 128], [1, N]])
            nc.scalar.dma_start(out=bias_sb[:], in_=bsrc)
            for q in (0, 2, 4, 6):
                x_big_dma(1, q, nc.scalar)
            for q in (1, 3, 5, 7):
                x_big_dma(1, q, nc.gpsimd)
            for q in (0, 2, 4, 6):
                x_big_dma(2, q, nc.scalar)
            for q in (1, 3, 5, 7):
                x_big_dma(2, q, nc.gpsimd)

            # ---- stores: quads of m-tiles, 8KB lines, partition-major out
            quad = {}   # ms0 -> o_sb tile

            def store_add(ms, po):
                """DVE bias-add of one m-tile into its quad tile; flush the
                quad DMA when the 4th slice lands. Last 4 m-tiles: singles."""
                if ms >= MT - 4:
                    o1 = osp.tile([128, N], dt.float32, tag="o1")
                    nc.vector.tensor_tensor(o1[:], po[:], bias_sb[:],
                                            mybir.AluOpType.add)
                    eng = nc.scalar if ms % 2 else nc.sync
                    if ms == MT - 1:
                        nc.sync.dma_start(
                            out=bass.AP(out[:, :].tensor, ms * N,
                                        [[MT * N, 64], [1, N]]),
                            in_=o1[0:64, :])
                        nc.scalar.dma_start(
                            out=bass.AP(out[:, :].tensor, 64 * MT * N + ms * N,
                                        [[MT * N, 64], [1, N]]),
                            in_=o1[64:128, :])
                    else:
                        eng.dma_start(out=out[:, ms * N:(ms + 1) * N],
                                      in_=o1[:])
                    return
                ms0 = (ms // 4) * 4
                if ms0 not in quad:
                    oq_t = oqp.tile([128, 4 * N], dt.float32, tag="oq")
                    quad[ms0] = oq_t
                o_sb = quad[ms0]
                sl = ms - ms0
                nc.vector.tensor_tensor(o_sb[:, sl * N:(sl + 1) * N],
                                        po[:], bias_sb[:],
                                        mybir.AluOpType.add)
                if sl == 3:
                    nc.sync.dma_start(out=out[:, ms0 * N:(ms0 + 4) * N],
                                      in_=o_sb[:])
                    del quad[ms0]

            def chase_superblock(g):
                """All 8 m-tiles of superblock g, k-blocks c-outer/j-inner."""
                po_g = []
                for _j in range(SB):
                    po = ps_out.tile([128, N], dt.float32, tag="po")
                    po_g.append(po)
                e = 0
                for b in KBLOCKS:
                    for c in range(e, e + b):
                        for j in range(SB):
                            nc.tensor.matmul(
                                po_g[j][:],
                                x_slice(g, c, j),
                                wt_sb[:, c * N:(c + 1) * N],
                                start=(c == 0), stop=(c == KC - 1),
                            )
                    e += b
                for j in range(SB):
                    store_add(g * SB + j, po_g[j])

            chase_superblock(0)
            chase_superblock(1)

            # ---- tail: k-inner m-tiles with full-superblock prefetch ----
            rr = [nc.scalar, nc.gpsimd]
            for ms in range(2 * SB, MT):
                g, j = ms // SB, ms % SB
                if g + 1 < NSB and (g + 1, j) not in big_tiles:
                    x_big_dma(g + 1, j, rr[j % 2])
                po = ps_out.tile([128, N], dt.float32, tag="po")
                for c in range(KC):
                    nc.tensor.matmul(
                        po[:],
                        x_slice(g, c, j),
                        wt_sb[:, c * N:(c + 1) * N],
                        start=(c == 0), stop=(c == KC - 1),
                    )
                store_add(ms, po)

    nc.compile()
    return nc


_cache = {}


def _get_program(lookup_table=None):
    # program is independent of input values
    if "nc" not in _cache:
        _cache["nc"] = build_program()
    return _cache["nc"]


def make_in_maps(inputs: dict):
    x = np.asarray(inputs["x"], dtype=np.float32).reshape(M, IN_F)
    xb = x.astype(ml_dtypes.bfloat16)
    # xt4[p, g, q, s, mm] = x[g*1024+mm, (q*XG+s)*128+p]
    xt4 = np.ascontiguousarray(
        xb.reshape(NSB, 1024, NQ, XG, 128).transpose(4, 0, 2, 3, 1)
    ).reshape(128, XROW)

    lut = np.asarray(inputs["lookup_table"], dtype=np.float32)
    pk_full = np.asarray(inputs["packed_qweight"]).astype(np.uint8).reshape(-1)
    idx_full = np.empty(pk_full.size * 2, np.uint8)
    idx_full[0::2] = pk_full & 15
    idx_full[1::2] = pk_full >> 4
    idx_full = idx_full.reshape(OUT_F, IN_F)

    wmax_full = np.asarray(inputs["weight_max"], dtype=np.float32).reshape(OUT_F, NBLK)
    lora_a = np.asarray(inputs["lora_A"], dtype=np.float32)
    lora_b = np.asarray(inputs["lora_B"], dtype=np.float32)
    bias_full = np.asarray(inputs["bias"], dtype=np.float32).reshape(-1)

    # full weight prep in f32, one rounding to bf16 at the end
    wf = lut[idx_full]                                    # [OUT_F, IN_F]
    wf = wf.reshape(OUT_F, NBLK, BLOCK) * wmax_full[:, :, None]
    wf = wf.reshape(OUT_F, IN_F) + lora_b @ lora_a        # LoRA fold

    in_maps = []
    for i in range(N_CORES):
        o0, o1 = i * N, (i + 1) * N
        wt = wf[o0:o1, :].T                               # [IN_F, N]
        weff = np.ascontiguousarray(
            wt.reshape(KC, 128, N).transpose(1, 0, 2).reshape(128, KC * N)
        ).astype(ml_dtypes.bfloat16)
        in_maps.append({
            "xt4": xt4,
            "weff": weff,
            "bias": bias_full[o0:o1],
        })
    return in_maps


def kernel(**inputs) -> np.ndarray:
    from concourse.bass_utils import run_bass_kernel_spmd

    nc = _get_program()
    in_maps = make_in_maps(inputs)
    res = run_bass_kernel_spmd(nc, in_maps, core_ids=list(range(N_CORES)))
    outs = []
    for r in res.results:
        o2 = np.asarray(r["out"], dtype=np.float32)       # [128, MT*N]
        outs.append(o2.reshape(128, MT, N).transpose(1, 0, 2).reshape(M, N))
    full = np.concatenate(outs, axis=1)  # [M, OUT_F]
    return full.reshape(B_, S_, OUT_F)
